# revision 16
# baseline (speedup 1.0000x reference)
"""ALiBi multi-head causal attention on 8 TRN2 NeuronCores.

Sharding: core = b*4 + hg  (b in 0..1 batches, hg in 0..3 head-groups).
Each core computes 4 heads of one batch end-to-end (KQV projection for its
head-columns + causal ALiBi attention).  No collectives needed.

Per-core kernel layout:
  - xT   [D, S]   : x[b].T (bf16)  -- contraction dim D on partitions
  - wKQ  [D, 1024]: W columns for K(h0..h3) then Q(h0..h3) (bf16)
  - wV   [D, 512] : W columns for V(h0..h3) (bf16)
  - kqT  = (x W_kq)^T computed as [hd, S] per head (head-dim on partitions)
  - v    = x W_v computed as [S, hd] blocks
  - scores  = qT-block^T @ kT  -> [128, L] in PSUM, + m*(t-i) bias tile
  - probs   = exp(score + (-m*128*qi)) (per-q-block shift makes bias m*(t-s))
              with accum_out giving the row sum (no max-subtraction; exponent
              is bounded: alibi bias <= 0 in the causal region, raw scores O(10))
  - PV: PE-transpose prob chunks, matmul with v blocks, accumulate in PSUM,
        then scale rows by 1/rowsum.
"""

import sys

if "/opt/trn_rl_repo" not in sys.path:
    sys.path.insert(0, "/opt/trn_rl_repo")

import numpy as np
import ml_dtypes

import concourse.bass as bass
import concourse.mybir as mybir
from concourse import bacc
from concourse.tile import TileContext
from concourse.masks import make_identity
from concourse.bass_utils import run_bass_kernel_spmd

P = 128
S = 2048
D = 2048
HD = 128
NB = S // P            # 16 seq blocks
H_LOC = 4              # heads per core
NUM_HEADS = 16
SCALE = 1.0 / np.sqrt(HD)

F32 = mybir.dt.float32
BF16 = mybir.dt.bfloat16
I32 = mybir.dt.int32
AF = mybir.ActivationFunctionType
OP = mybir.AluOpType


def _alibi_slopes(num_heads=NUM_HEADS):
    base = (2.0 ** 8) ** (1.0 / num_heads)
    return np.asarray([1.0 / base ** (i + 1) for i in range(num_heads)], np.float32)


def build():
    nc = bacc.Bacc("TRN2", target_bir_lowering=False)

    xT_d = nc.declare_dram_parameter("xT", [D, S], BF16, isOutput=False)
    wKQ_d = nc.declare_dram_parameter("wKQ", [D, 8 * P], BF16, isOutput=False)
    wV_d = nc.declare_dram_parameter("wV", [D, H_LOC * HD], BF16, isOutput=False)
    bKQ_d = nc.declare_dram_parameter("bKQ", [P, 8], F32, isOutput=False)
    bVT_d = nc.declare_dram_parameter("bVT", [HD, H_LOC], F32, isOutput=False)
    biasH_d = nc.declare_dram_parameter("biasH", [H_LOC, P, S], F32, isOutput=False)
    negsh_d = nc.declare_dram_parameter("negsh", [P, H_LOC, NB], F32, isOutput=False)
    causal_d = nc.declare_dram_parameter("causal", [P, P], F32, isOutput=False)
    # out in transposed-per-head layout [head, hd, s]; host transposes back
    out_d = nc.declare_dram_parameter("out", [H_LOC, HD, S], F32, isOutput=True)

    xT_t = xT_d.rearrange("(ko p) s -> p ko s", p=P)     # [128, 16, 2048]
    wKQ_t = wKQ_d.rearrange("(ko p) n -> p ko n", p=P)   # [128, 16, 1024]
    wV_t = wV_d.rearrange("(ko p) n -> p ko n", p=P)     # [128, 16, 512]

    with TileContext(nc) as tc:
        with (
            tc.tile_pool(name="const", bufs=1) as const,
            tc.tile_pool(name="resid", bufs=1) as resid,
            tc.tile_pool(name="stats", bufs=4) as stats,
            tc.tile_pool(name="psA", bufs=4, space="PSUM") as psA,
            tc.tile_pool(name="psO", bufs=2, space="PSUM") as psO,
            tc.tile_pool(name="psR", bufs=1, space="PSUM") as psR,
        ):
            # ---- constants ----
            ident_f = const.tile([P, P], F32)
            make_identity(nc, ident_f)

            causal = const.tile([P, P], F32)  # additive: 0 if t<=s else -1e30
            nc.sync.dma_start(causal, causal_d[:])

            bkq_sb = const.tile([P, 8], F32)
            nc.sync.dma_start(bkq_sb, bKQ_d[:])
            bvt_sb = const.tile([HD, H_LOC], F32)
            nc.sync.dma_start(bvt_sb, bVT_d[:])

            neg_shift = const.tile([P, H_LOC, NB], F32)  # -m_h * 128 * qi
            nc.sync.dma_start(neg_shift, negsh_d[:])

            # ---- residents ----
            kq_all = resid.tile([P, 8, S], BF16)       # [hd, (K h0..3 | Q h0..3), s]
            v_all = resid.tile([P, NB, H_LOC * HD], BF16)  # [si, so, j*128+d]

            # ---- phase 1: KQV projection ----
            with (
                tc.tile_pool(name="wpool", bufs=1) as wpool,
                tc.tile_pool(name="xpool", bufs=2) as xpool,
            ):
                wkq_sb = wpool.tile([P, 16, 8 * P], BF16)
                nc.sync.dma_start(wkq_sb, wKQ_t)
                wv_sb = wpool.tile([P, 16, H_LOC * HD], BF16)
                nc.sync.dma_start(wv_sb, wV_t)

                for nb in range(S // 512):
                    xc = xpool.tile([P, 16, 512], BF16, tag="xc")
                    nc.sync.dma_start(xc, xT_t[:, :, nb * 512 : (nb + 1) * 512])
                    for m in range(8):
                        ps = psA.tile([P, 512], F32, tag="ps")
                        for k in range(16):
                            nc.tensor.matmul(
                                ps,
                                lhsT=wkq_sb[:, k, m * P : (m + 1) * P],
                                rhs=xc[:, k, :],
                                start=(k == 0),
                                stop=(k == 15),
                            )
                        # kqT = psum * scale + bias  (scale folds 1/sqrt(hd) into q)
                        nc.scalar.activation(
                            kq_all[:, m, nb * 512 : (nb + 1) * 512],
                            ps,
                            AF.Identity,
                            bias=bkq_sb[:, m : m + 1],
                            scale=float(SCALE) if m >= 4 else 1.0,
                        )
                    for sub in range(4):
                        s_idx = nb * 4 + sub
                        psv = psA.tile([P, 512], F32, tag="ps")
                        for k in range(16):
                            nc.tensor.matmul(
                                psv,
                                lhsT=xc[:, k, sub * P : (sub + 1) * P],
                                rhs=wv_sb[:, k, :],
                                start=(k == 0),
                                stop=(k == 15),
                            )
                        nc.vector.tensor_copy(v_all[:, s_idx, :], psv)

            # ---- phase 2: attention (groups of 4 q-blocks) ----
            with (
                tc.tile_pool(name="attn", bufs=2) as attn_pool,
                tc.tile_pool(name="probsp", bufs=3) as probs_pool,
                tc.tile_pool(name="biasp", bufs=2) as bias_pool,
            ):
                for j in range(H_LOC):
                    bias_h = bias_pool.tile([P, S], F32, tag="bias_h")
                    nc.sync.dma_start(bias_h, biasH_d[j])
                    for G in range(NB // 4):
                        last_c = 4 * G + 3
                        # pt[t, c, group_col]: transposed probs for the group
                        pt = attn_pool.tile([P, NB, 512], BF16, tag="pt")
                        # zero non-causal regions of partial chunks
                        for c in range(4 * G + 1, last_c + 1):
                            nc.any.memzero(pt[:, c, : (c - 4 * G) * P])
                        rowsum4 = stats.tile([P, 4], F32, tag="rowsum4")
                        for ql in range(4):
                            qi = 4 * G + ql
                            L = (qi + 1) * P
                            nch = (L + 511) // 512
                            score = attn_pool.tile([P, S], F32, tag="score")
                            for c5 in range(nch):
                                w = min(512, L - c5 * 512)
                                ps = psA.tile([P, 512], F32, tag="ps")
                                nc.tensor.matmul(
                                    ps[:, :w],
                                    lhsT=kq_all[:, 4 + j, qi * P : (qi + 1) * P],
                                    rhs=kq_all[:, j, c5 * 512 : c5 * 512 + w],
                                    start=True,
                                    stop=True,
                                )
                                nc.vector.tensor_tensor(
                                    score[:, c5 * 512 : c5 * 512 + w],
                                    ps[:, :w],
                                    bias_h[:, c5 * 512 : c5 * 512 + w],
                                    OP.add,
                                )
                            # causal mask on the diagonal block
                            nc.vector.tensor_tensor(
                                score[:, qi * P : L],
                                score[:, qi * P : L],
                                causal,
                                OP.add,
                            )
                            probs = probs_pool.tile([P, S], BF16, tag="probs")
                            rspart = stats.tile([P, 4], F32, tag="rspart")
                            for c5 in range(nch):
                                w = min(512, L - c5 * 512)
                                nc.scalar.activation(
                                    probs[:, c5 * 512 : c5 * 512 + w],
                                    score[:, c5 * 512 : c5 * 512 + w],
                                    AF.Exp,
                                    bias=neg_shift[:, j, qi : qi + 1],
                                    scale=1.0,
                                    accum_out=rspart[:, c5 : c5 + 1],
                                )
                            nc.vector.reduce_sum(
                                rowsum4[:, ql : ql + 1],
                                rspart[:, :nch],
                                axis=mybir.AxisListType.X,
                            )
                            # XBAR (DMA) transposes of prob chunks into pt
                            for c in range(qi + 1):
                                nc.scalar.dma_start(
                                    pt[:, c, ql * P : (ql + 1) * P],
                                    probs[:, c * P : (c + 1) * P],
                                    transpose=True,
                                )
                        # PV: outT[hd, sq_group] accumulated over t-chunks
                        po = psO.tile([P, 512], F32, tag="po")
                        for c in range(last_c + 1):
                            nc.tensor.matmul(
                                po,
                                lhsT=v_all[:, c, j * HD : (j + 1) * HD],
                                rhs=pt[:, c, :],
                                start=(c == 0),
                                stop=(c == last_c),
                            )
        # broadcast-transpose the per-row reciprocals across partitions:
                        # transpose(recip4[:, ql] broadcast along free) puts
                        # recip[sq] into every partition row of rb[:, sq]
                        recip4 = stats.tile([P, 4], F32, tag="recip4")
                        nc.vector.reciprocal(recip4, rowsum4)
                        rb = psR.tile([P, 512], F32, tag="rb")
                        for ql in range(4):
                            nc.tensor.transpose(
                                rb[:, ql * P : (ql + 1) * P],
                                recip4[:, ql : ql + 1].to_broadcast((P, P)),
                                ident_f,
                            )
                        rb_sb = attn_pool.tile([P, 512], F32, tag="rbsb")
                        nc.vector.tensor_copy(rb_sb, rb)
                        out_sb = attn_pool.tile([P, 512], F32, tag="osb")
                        nc.vector.tensor_tensor(out_sb, po, rb_sb, OP.mult)
                        # + V-projection bias (sum of normalized probs == 1)
                        nc.scalar.activation(
                            out_sb,
                            out_sb,
                            AF.Identity,
                            bias=bvt_sb[:, j : j + 1],
                            scale=1.0,
                        )
                        nc.sync.dma_start(
                            out_d[j][:, G * 512 : (G + 1) * 512], out_sb
                        )

    nc.finalize()
    return nc


_NC_CACHE = None


def _get_nc():
    global _NC_CACHE
    if _NC_CACHE is None:
        _NC_CACHE = build()
    return _NC_CACHE


def _make_in_maps(x, W_kqv, b_kqv):
    x = np.asarray(x, np.float32)
    W = np.asarray(W_kqv, np.float32)
    b = np.asarray(b_kqv, np.float32)
    slopes = _alibi_slopes()
    in_maps = []
    for core in range(8):
        bi, hg = divmod(core, 4)
        heads = [4 * hg + j for j in range(H_LOC)]
        xT = np.ascontiguousarray(x[bi].T).astype(ml_dtypes.bfloat16)
        wkq = np.concatenate(
            [W[:, h * HD : (h + 1) * HD] for h in heads]
            + [W[:, D + h * HD : D + (h + 1) * HD] for h in heads],
            axis=1,
        ).astype(ml_dtypes.bfloat16)
        wv = np.concatenate(
            [W[:, 2 * D + h * HD : 2 * D + (h + 1) * HD] for h in heads], axis=1
        ).astype(ml_dtypes.bfloat16)
        # bias columns: K h0..h3 then Q h0..h3; q-side prescaled by 1/sqrt(hd)
        bkq = np.stack(
            [b[h * HD : (h + 1) * HD] for h in heads]
            + [b[D + h * HD : D + (h + 1) * HD] * SCALE for h in heads],
            axis=1,
        ).astype(np.float32)
        bvt = np.stack(
            [b[2 * D + h * HD : 2 * D + (h + 1) * HD] for h in heads], axis=1
        ).astype(np.float32)  # [hd, H_LOC]
        # biasH[j, i, t] = m_h * (t - i);  negsh[p, j, qi] = -m_h * 128 * qi
        rel = (np.arange(S)[None, :] - np.arange(P)[:, None]).astype(np.float32)
        bias_h = (slopes[heads][:, None, None] * rel[None]).astype(np.float32)
        negsh = np.tile(
            (-slopes[heads][:, None] * (P * np.arange(NB))[None, :])[None],
            (P, 1, 1),
        ).astype(np.float32)
        causal = np.where(
            np.arange(P)[:, None] >= np.arange(P)[None, :], 0.0, -1e30
        ).astype(np.float32)
        in_maps.append(
            dict(
                xT=xT, wKQ=wkq, wV=wv, bKQ=bkq, bVT=bvt,
                biasH=bias_h, negsh=negsh, causal=causal,
            )
        )
    return in_maps


def run(inputs, trace=False, **kw):
    nc = _get_nc()
    in_maps = _make_in_maps(inputs["x"], inputs["W_kqv"], inputs["b_kqv"])
    bkr = run_bass_kernel_spmd(nc, in_maps, core_ids=list(range(8)), trace=trace, **kw)
    B = 2
    out = np.empty((B, NUM_HEADS, S, HD), np.float32)
    for core in range(8):
        bi, hg = divmod(core, 4)
        o = np.asarray(bkr.results[core]["out"])  # [4, 128(hd), 2048(s)]
        for j in range(H_LOC):
            out[bi, 4 * hg + j] = o[j].T
    return out, bkr


def kernel(x, W_kqv, b_kqv):
    out, _ = run({"x": x, "W_kqv": W_kqv, "b_kqv": b_kqv})
    return out


# revision 24
# speedup vs baseline: 2.3524x; 2.3524x over previous
"""ALiBi multi-head causal attention on 8 TRN2 NeuronCores.

Sharding: core = b*4 + hg  (b in 0..1 batches, hg in 0..3 head-groups).
Each core computes 4 heads of one batch end-to-end (KQV projection for its
head-columns + causal ALiBi attention).  No collectives needed.

Per-core kernel layout:
  - xT   [D, S]   : x[b].T (bf16)  -- contraction dim D on partitions
  - wKQ  [D, 1024]: W columns for K(h0..h3) then Q(h0..h3) (bf16)
  - wV   [D, 512] : W columns for V(h0..h3) (bf16)
  - kqT  = (x W_kq)^T computed as [hd, S] per head (head-dim on partitions)
  - v    = x W_v computed as [S, hd] blocks
  - scores  = qT-block^T @ kT  -> [128, L] in PSUM, + m*(t-i) bias tile
  - probs   = exp(score + (-m*128*qi)) (per-q-block shift makes bias m*(t-s))
              with accum_out giving the row sum (no max-subtraction; exponent
              is bounded: alibi bias <= 0 in the causal region, raw scores O(10))
  - PV: PE-transpose prob chunks, matmul with v blocks, accumulate in PSUM,
        then scale rows by 1/rowsum.
"""

import sys

if "/opt/trn_rl_repo" not in sys.path:
    sys.path.insert(0, "/opt/trn_rl_repo")

import numpy as np
import ml_dtypes

import concourse.bass as bass
import concourse.mybir as mybir
from concourse import bacc
from concourse.tile import TileContext
from concourse.masks import make_identity
from concourse.bass_utils import run_bass_kernel_spmd

P = 128
S = 2048
D = 2048
HD = 128
NB = S // P            # 16 seq blocks
H_LOC = 4              # heads per core
NUM_HEADS = 16
SCALE = 1.0 / np.sqrt(HD)

F32 = mybir.dt.float32
F32R = mybir.dt.float32r
BF16 = mybir.dt.bfloat16
I32 = mybir.dt.int32
AF = mybir.ActivationFunctionType
OP = mybir.AluOpType


def _alibi_slopes(num_heads=NUM_HEADS):
    base = (2.0 ** 8) ** (1.0 / num_heads)
    return np.asarray([1.0 / base ** (i + 1) for i in range(num_heads)], np.float32)


def build():
    nc = bacc.Bacc("TRN2", target_bir_lowering=False)

    xT_d = nc.declare_dram_parameter("xT", [D, S], BF16, isOutput=False)
    wKQ_d = nc.declare_dram_parameter("wKQ", [D, 8 * P], BF16, isOutput=False)
    wV_d = nc.declare_dram_parameter("wV", [D, H_LOC * HD], BF16, isOutput=False)
    bKQ_d = nc.declare_dram_parameter("bKQ", [P, 8], F32, isOutput=False)
    bVT_d = nc.declare_dram_parameter("bVT", [HD, H_LOC], F32, isOutput=False)
    # transposed-space bias: biasT[j, tl, sqg] = m_j * (tl - sqg)
    biasT_d = nc.declare_dram_parameter("biasT", [H_LOC, P, 512], F32, isOutput=False)
    # per-chunk shift: negshT[p, j, d+12] = m_j * 128 * d   (d = c - 4G)
    negshT_d = nc.declare_dram_parameter("negshT", [P, H_LOC, 16], F32, isOutput=False)
    # transposed causal: 0 if tl <= sql else -1e30
    causalT_d = nc.declare_dram_parameter("causalT", [P, P], F32, isOutput=False)
    # out in transposed-per-head layout [head, hd, s]; host transposes back
    out_d = nc.declare_dram_parameter("out", [H_LOC, HD, S], F32, isOutput=True)

    xT_t = xT_d.rearrange("(ko p) s -> p ko s", p=P)     # [128, 16, 2048]
    wKQ_t = wKQ_d.rearrange("(ko p) n -> p ko n", p=P)   # [128, 16, 1024]
    wV_t = wV_d.rearrange("(ko p) n -> p ko n", p=P)     # [128, 16, 512]

    with TileContext(nc) as tc:
        with (
            tc.tile_pool(name="const", bufs=1) as const,
            tc.tile_pool(name="resid", bufs=1) as resid,
            tc.tile_pool(name="stats", bufs=4) as stats,
            tc.tile_pool(name="psA", bufs=3, space="PSUM") as psA,
            tc.tile_pool(name="psO", bufs=2, space="PSUM") as psO,
            tc.tile_pool(name="psR", bufs=1, space="PSUM") as psR,
        ):
            # ---- constants ----
            causalT = const.tile([P, P], F32)
            nc.sync.dma_start(causalT, causalT_d[:])

            bkq_sb = const.tile([P, 8], F32)
            nc.sync.dma_start(bkq_sb, bKQ_d[:])
            bvt_sb = const.tile([HD, H_LOC], F32)
            nc.sync.dma_start(bvt_sb, bVT_d[:])

            negshT = const.tile([P, H_LOC, 16], F32)
            nc.sync.dma_start(negshT, negshT_d[:])

            ones_bf = const.tile([P, 1], BF16)  # rowsum column
            nc.gpsimd.memset(ones_bf, 1.0)
            ones1_raw = const.tile([1, P], F32)
            nc.gpsimd.memset(ones1_raw, 1.0)
            ones1_f = const.tile([1, P], F32R)  # partition-broadcast row
            with nc.allow_low_precision(reason="constant ones cast to f32r"):
                nc.vector.tensor_copy(ones1_f, ones1_raw)

            # ---- residents ----
            kq_all = resid.tile([P, 8, S], BF16)       # [hd, (K h0..3 | Q h0..3), s]
            v_all = resid.tile([P, NB, H_LOC * HD], BF16)  # [si, so, j*128+d]

            # ---- phase 1: KQV projection ----
            with (
                tc.tile_pool(name="wpool", bufs=1) as wpool,
                tc.tile_pool(name="xpool", bufs=2) as xpool,
            ):
                wkq_sb = wpool.tile([P, 16, 8 * P], BF16)
                nc.sync.dma_start(wkq_sb, wKQ_t)
                wv_sb = wpool.tile([P, 16, H_LOC * HD], BF16)
                nc.sync.dma_start(wv_sb, wV_t)

                for nb in range(S // 512):
                    xc = xpool.tile([P, 16, 512], BF16, tag="xc")
                    nc.sync.dma_start(xc, xT_t[:, :, nb * 512 : (nb + 1) * 512])
                    for m in range(8):
                        ps = psA.tile([P, 512], F32, tag="ps")
                        for k in range(16):
                            nc.tensor.matmul(
                                ps,
                                lhsT=wkq_sb[:, k, m * P : (m + 1) * P],
                                rhs=xc[:, k, :],
                                start=(k == 0),
                                stop=(k == 15),
                            )
                        # kqT = psum * scale + bias  (scale folds 1/sqrt(hd) into q)
                        nc.scalar.activation(
                            kq_all[:, m, nb * 512 : (nb + 1) * 512],
                            ps,
                            AF.Identity,
                            bias=bkq_sb[:, m : m + 1],
                            scale=float(SCALE) if m >= 4 else 1.0,
                        )
                    for sub in range(4):
                        s_idx = nb * 4 + sub
                        psv = psA.tile([P, 512], F32, tag="ps")
                        for k in range(16):
                            nc.tensor.matmul(
                                psv,
                                lhsT=xc[:, k, sub * P : (sub + 1) * P],
                                rhs=wv_sb[:, k, :],
                                start=(k == 0),
                                stop=(k == 15),
                            )
                        nc.vector.tensor_copy(v_all[:, s_idx, :], psv)

            # ---- phase 2: attention, transposed score space ----
            # scoreT[t, sq]: k stationary, q-group moving (N=512).  Softmax
            # needs only elementwise ops (bias/mask/exp) + a partition-axis
            # rowsum, done as an M=1 ones-matmul.  PV consumes probsT
            # directly -- no per-chunk transposes anywhere.
            with (
                tc.tile_pool(name="attn", bufs=2) as attn_pool,
                tc.tile_pool(name="biasp", bufs=2) as bias_pool,
                tc.tile_pool(name="psS", bufs=2, space="PSUM") as psS,
            ):
                for j in range(H_LOC):
                    biasT = bias_pool.tile([P, 512], F32, tag="biasT")
                    nc.sync.dma_start(biasT, biasT_d[j])
                    for G in range(NB // 4):
                        last_c = 4 * G + 3
                        # probsT[t, c, group_col]
                        probsT = attn_pool.tile([P, NB, 512], BF16, tag="pT")
                        rs_ps = psS.tile([1, 512], F32, tag="rs")
                        for c in range(last_c + 1):
                            d = c - 4 * G  # -12..3
                            ps = psA.tile([P, 512], F32, tag="ps")
                            nc.tensor.matmul(
                                ps,
                                lhsT=kq_all[:, j, c * P : (c + 1) * P],
                                rhs=kq_all[:, 4 + j, G * 512 : (G + 1) * 512],
                                start=True,
                                stop=True,
                            )
                            scoreT = attn_pool.tile([P, 512], F32, tag="scT")
                            nc.vector.tensor_tensor(scoreT, ps, biasT, OP.add)
                            if d >= 0:
                                nc.vector.tensor_tensor(
                                    scoreT[:, d * P : (d + 1) * P],
                                    scoreT[:, d * P : (d + 1) * P],
                                    causalT,
                                    OP.add,
                                )
                            nc.scalar.activation(
                                probsT[:, c, :],
                                scoreT,
                                AF.Exp,
                                bias=negshT[:, j, d + 12 : d + 13],
                                scale=1.0,
                            )
                            if d > 0:
                                # fully non-causal sub-blocks (exp may be inf)
                                nc.any.memzero(probsT[:, c, : d * P])
                            nc.tensor.matmul(
                                rs_ps,
                                lhsT=ones_bf,
                                rhs=probsT[:, c, :],
                                start=(c == 0),
                                stop=(c == last_c),
                            )
                        # PV: outT[hd, sq_group] accumulated over t-chunks
                        po = psO.tile([P, 512], F32, tag="po")
                        for c in range(last_c + 1):
                            nc.tensor.matmul(
                                po,
                                lhsT=v_all[:, c, j * HD : (j + 1) * HD],
                                rhs=probsT[:, c, :],
                                start=(c == 0),
                                stop=(c == last_c),
                            )
                        # normalize: rb = ones1^T @ recip(rowsumT), bcast over
                        # partitions via K=1 f32r matmul
                        rs_sb = stats.tile([1, 512], F32R, tag="rs_sb")
                        with nc.allow_low_precision(
                            reason="f32r == f32 bit layout; rounding only"
                        ):
                            nc.vector.reciprocal(rs_sb, rs_ps)
                        rb = psR.tile([P, 512], F32, tag="rb")
                        nc.tensor.matmul(
                            rb,
                            lhsT=ones1_f,
                            rhs=rs_sb,
                            start=True,
                            stop=True,
                        )
                        rb_sb = attn_pool.tile([P, 512], F32, tag="rbsb")
                        nc.vector.tensor_copy(rb_sb, rb)
                        out_sb = attn_pool.tile([P, 512], F32, tag="osb")
                        nc.vector.tensor_tensor(out_sb, po, rb_sb, OP.mult)
                        # + V-projection bias (sum of normalized probs == 1)
                        nc.scalar.activation(
                            out_sb,
                            out_sb,
                            AF.Identity,
                            bias=bvt_sb[:, j : j + 1],
                            scale=1.0,
                        )
                        nc.sync.dma_start(
                            out_d[j][:, G * 512 : (G + 1) * 512], out_sb
                        )

    nc.finalize()
    return nc


_NC_CACHE = None


def _get_nc():
    global _NC_CACHE
    if _NC_CACHE is None:
        _NC_CACHE = build()
    return _NC_CACHE


def _make_in_maps(x, W_kqv, b_kqv):
    x = np.asarray(x, np.float32)
    W = np.asarray(W_kqv, np.float32)
    b = np.asarray(b_kqv, np.float32)
    slopes = _alibi_slopes()
    in_maps = []
    for core in range(8):
        bi, hg = divmod(core, 4)
        heads = [4 * hg + j for j in range(H_LOC)]
        xT = np.ascontiguousarray(x[bi].T).astype(ml_dtypes.bfloat16)
        wkq = np.concatenate(
            [W[:, h * HD : (h + 1) * HD] for h in heads]
            + [W[:, D + h * HD : D + (h + 1) * HD] for h in heads],
            axis=1,
        ).astype(ml_dtypes.bfloat16)
        wv = np.concatenate(
            [W[:, 2 * D + h * HD : 2 * D + (h + 1) * HD] for h in heads], axis=1
        ).astype(ml_dtypes.bfloat16)
        # bias columns: K h0..h3 then Q h0..h3; q-side prescaled by 1/sqrt(hd)
        bkq = np.stack(
            [b[h * HD : (h + 1) * HD] for h in heads]
            + [b[D + h * HD : D + (h + 1) * HD] * SCALE for h in heads],
            axis=1,
        ).astype(np.float32)
        bvt = np.stack(
            [b[2 * D + h * HD : 2 * D + (h + 1) * HD] for h in heads], axis=1
        ).astype(np.float32)  # [hd, H_LOC]
        # biasT[j, tl, sqg] = m_j * (tl - sqg)
        relT = (np.arange(P)[:, None] - np.arange(512)[None, :]).astype(np.float32)
        bias_t = (slopes[heads][:, None, None] * relT[None]).astype(np.float32)
        # negshT[p, j, d+12] = m_j * 128 * d, d in [-12, 3]
        dvals = (np.arange(16) - 12).astype(np.float32) * P
        negsht = np.tile(
            (slopes[heads][:, None] * dvals[None, :])[None], (P, 1, 1)
        ).astype(np.float32)
        # transposed causal: keep tl <= sql
        causalt = np.where(
            np.arange(P)[:, None] <= np.arange(P)[None, :], 0.0, -1e30
        ).astype(np.float32)
        in_maps.append(
            dict(
                xT=xT, wKQ=wkq, wV=wv, bKQ=bkq, bVT=bvt,
                biasT=bias_t, negshT=negsht, causalT=causalt,
            )
        )
    return in_maps


def run(inputs, trace=False, **kw):
    nc = _get_nc()
    in_maps = _make_in_maps(inputs["x"], inputs["W_kqv"], inputs["b_kqv"])
    bkr = run_bass_kernel_spmd(nc, in_maps, core_ids=list(range(8)), trace=trace, **kw)
    B = 2
    out = np.empty((B, NUM_HEADS, S, HD), np.float32)
    for core in range(8):
        bi, hg = divmod(core, 4)
        o = np.asarray(bkr.results[core]["out"])  # [4, 128(hd), 2048(s)]
        for j in range(H_LOC):
            out[bi, 4 * hg + j] = o[j].T
    return out, bkr


def kernel(x, W_kqv, b_kqv):
    out, _ = run({"x": x, "W_kqv": W_kqv, "b_kqv": b_kqv})
    return out


# revision 29
# speedup vs baseline: 2.4280x; 1.0321x over previous
"""ALiBi multi-head causal attention on 8 TRN2 NeuronCores.

Sharding: core = b*4 + hg  (b in 0..1 batches, hg in 0..3 head-groups).
Each core computes 4 heads of one batch end-to-end (KQV projection for its
head-columns + causal ALiBi attention).  No collectives needed.

Per-core kernel layout:
  - xT   [D, S]   : x[b].T (bf16)  -- contraction dim D on partitions
  - wKQ  [D, 1024]: W columns for K(h0..h3) then Q(h0..h3) (bf16)
  - wV   [D, 512] : W columns for V(h0..h3) (bf16)
  - kqT  = (x W_kq)^T computed as [hd, S] per head (head-dim on partitions)
  - v    = x W_v computed as [S, hd] blocks
  - scores  = qT-block^T @ kT  -> [128, L] in PSUM, + m*(t-i) bias tile
  - probs   = exp(score + (-m*128*qi)) (per-q-block shift makes bias m*(t-s))
              with accum_out giving the row sum (no max-subtraction; exponent
              is bounded: alibi bias <= 0 in the causal region, raw scores O(10))
  - PV: PE-transpose prob chunks, matmul with v blocks, accumulate in PSUM,
        then scale rows by 1/rowsum.
"""

import sys

if "/opt/trn_rl_repo" not in sys.path:
    sys.path.insert(0, "/opt/trn_rl_repo")

import numpy as np
import ml_dtypes

import concourse.bass as bass
import concourse.mybir as mybir
from concourse import bacc
from concourse.tile import TileContext
from concourse.masks import make_identity
from concourse.bass_utils import run_bass_kernel_spmd

P = 128
S = 2048
D = 2048
HD = 128
NB = S // P            # 16 seq blocks
H_LOC = 4              # heads per core
NUM_HEADS = 16
SCALE = 1.0 / np.sqrt(HD)

F32 = mybir.dt.float32
F32R = mybir.dt.float32r
BF16 = mybir.dt.bfloat16
I32 = mybir.dt.int32
AF = mybir.ActivationFunctionType
OP = mybir.AluOpType


def _alibi_slopes(num_heads=NUM_HEADS):
    base = (2.0 ** 8) ** (1.0 / num_heads)
    return np.asarray([1.0 / base ** (i + 1) for i in range(num_heads)], np.float32)


def build():
    nc = bacc.Bacc("TRN2", target_bir_lowering=False)

    xT_d = nc.declare_dram_parameter("xT", [D, S], BF16, isOutput=False)
    wKQ_d = nc.declare_dram_parameter("wKQ", [D, 8 * P], BF16, isOutput=False)
    wV_d = nc.declare_dram_parameter("wV", [D, H_LOC * HD], BF16, isOutput=False)
    bKQ_d = nc.declare_dram_parameter("bKQ", [P, 8], F32, isOutput=False)
    bVT_d = nc.declare_dram_parameter("bVT", [HD, H_LOC], F32, isOutput=False)
    # transposed-space bias: biasT[j, tl, sqg] = m_j * (tl - sqg)
    biasT_d = nc.declare_dram_parameter("biasT", [H_LOC, P, 512], F32, isOutput=False)
    # per-chunk shift: negshT[p, j, d+12] = m_j * 128 * d   (d = c - 4G)
    negshT_d = nc.declare_dram_parameter("negshT", [P, H_LOC, 16], F32, isOutput=False)
    # transposed causal: 0 if tl <= sql else -1e30
    causalT_d = nc.declare_dram_parameter("causalT", [P, P], F32, isOutput=False)
    # out in transposed-per-head layout [head, hd, s]; host transposes back
    out_d = nc.declare_dram_parameter("out", [H_LOC, HD, S], F32, isOutput=True)

    xT_t = xT_d.rearrange("(ko p) s -> p ko s", p=P)     # [128, 16, 2048]
    wKQ_t = wKQ_d.rearrange("(ko p) n -> p ko n", p=P)   # [128, 16, 1024]
    wV_t = wV_d.rearrange("(ko p) n -> p ko n", p=P)     # [128, 16, 512]

    with TileContext(nc) as tc:
        with (
            tc.tile_pool(name="const", bufs=1) as const,
            tc.tile_pool(name="resid", bufs=1) as resid,
            tc.tile_pool(name="stats", bufs=4) as stats,
            tc.tile_pool(name="psA", bufs=3, space="PSUM") as psA,
            tc.tile_pool(name="psO", bufs=2, space="PSUM") as psO,
            tc.tile_pool(name="psS", bufs=2, space="PSUM") as psS,
            tc.tile_pool(name="wpool", bufs=1) as wpool,
            tc.tile_pool(name="xpool", bufs=2) as xpool,
            tc.tile_pool(name="attn", bufs=2) as attn_pool,
            tc.tile_pool(name="biasp", bufs=2) as bias_pool,
        ):
            # ---- constants ----
            causalT = const.tile([P, P], F32)
            nc.sync.dma_start(causalT, causalT_d[:])

            bkq_sb = const.tile([P, 8], F32)
            nc.sync.dma_start(bkq_sb, bKQ_d[:])
            bvt_sb = const.tile([HD, H_LOC], F32)
            nc.sync.dma_start(bvt_sb, bVT_d[:])

            negshT = const.tile([P, H_LOC, 16], F32)
            nc.sync.dma_start(negshT, negshT_d[:])

            ones_bf = const.tile([P, 1], BF16)  # rowsum column
            nc.gpsimd.memset(ones_bf, 1.0)
            ones1_raw = const.tile([1, P], F32)
            nc.gpsimd.memset(ones1_raw, 1.0)
            ones1_f = const.tile([1, P], F32R)  # partition-broadcast row
            with nc.allow_low_precision(reason="constant ones cast to f32r"):
                nc.vector.tensor_copy(ones1_f, ones1_raw)

            # ---- residents ----
            kq_all = resid.tile([P, 8, S], BF16)       # [hd, (K h0..3 | Q h0..3), s]
            v_all = resid.tile([P, NB, H_LOC * HD], BF16)  # [si, so, j*128+d]

            # ---- phase 1: KQV projection ----
            wkq_sb = wpool.tile([P, 16, 8 * P], BF16)
            for k in range(16):
                nc.sync.dma_start(wkq_sb[:, k, :], wKQ_t[:, k, :])
            wv_sb = wpool.tile([P, 16, H_LOC * HD], BF16)
            for k in range(16):
                nc.sync.dma_start(wv_sb[:, k, :], wV_t[:, k, :])

            for nb in range(S // 512):
                xc = xpool.tile([P, 16, 512], BF16, tag="xc")
                for k in range(16):
                    nc.sync.dma_start(
                        xc[:, k, :], xT_t[:, k, nb * 512 : (nb + 1) * 512]
                    )
                for m in range(8):
                    ps = psA.tile([P, 512], F32, tag="ps")
                    for k in range(16):
                        nc.tensor.matmul(
                            ps,
                            lhsT=wkq_sb[:, k, m * P : (m + 1) * P],
                            rhs=xc[:, k, :],
                            start=(k == 0),
                            stop=(k == 15),
                        )
                    # kqT = psum * scale + bias (scale folds 1/sqrt(hd) into q)
                    nc.scalar.activation(
                        kq_all[:, m, nb * 512 : (nb + 1) * 512],
                        ps,
                        AF.Identity,
                        bias=bkq_sb[:, m : m + 1],
                        scale=float(SCALE) if m >= 4 else 1.0,
                    )
                for sub in range(4):
                    s_idx = nb * 4 + sub
                    psv = psA.tile([P, 512], F32, tag="ps")
                    for k in range(16):
                        nc.tensor.matmul(
                            psv,
                            lhsT=xc[:, k, sub * P : (sub + 1) * P],
                            rhs=wv_sb[:, k, :],
                            start=(k == 0),
                            stop=(k == 15),
                        )
                    nc.vector.tensor_copy(v_all[:, s_idx, :], psv)

            # ---- phase 2: attention, transposed score space ----
            # scoreT[t, sq]: k stationary, q-group moving (N=512).  Softmax
            # needs only elementwise ops (bias/mask/exp) + a partition-axis
            # rowsum (M=1 ones-matmul).  PV consumes probsT directly -- no
            # per-chunk transposes anywhere.  Only the causally-valid column
            # range [lo:512] of each chunk is computed; the rest is zeroed.
            for j in range(H_LOC):
                biasT = bias_pool.tile([P, 512], F32, tag="biasT")
                nc.sync.dma_start(biasT, biasT_d[j])
                for G in range(NB // 4):
                    last_c = 4 * G + 3
                    # probsT[t, c, group_col]
                    probsT = attn_pool.tile([P, NB, 512], BF16, tag="pT")
                    rs_ps = psS.tile([1, 512], F32, tag="rs")
                    for c in range(last_c + 1):
                        d = c - 4 * G  # -12..3
                        lo = max(0, d) * P  # first causally-valid column
                        if lo > 0:
                            nc.vector.memset(probsT[:, c, :lo], 0.0)
                        ps = psA.tile([P, 512], F32, tag="ps")
                        nc.tensor.matmul(
                            ps,
                            lhsT=kq_all[:, j, c * P : (c + 1) * P],
                            rhs=kq_all[:, 4 + j, G * 512 : (G + 1) * 512],
                            start=True,
                            stop=True,
                        )
                        scoreT = attn_pool.tile([P, 512], F32, tag="scT")
                        nc.vector.tensor_tensor(
                            scoreT[:, lo:], ps[:, lo:], biasT[:, lo:], OP.add
                        )
                        if d >= 0:
                            nc.vector.tensor_tensor(
                                scoreT[:, d * P : (d + 1) * P],
                                scoreT[:, d * P : (d + 1) * P],
                                causalT,
                                OP.add,
                            )
                        nc.scalar.activation(
                            probsT[:, c, lo:],
                            scoreT[:, lo:],
                            AF.Exp,
                            bias=negshT[:, j, d + 12 : d + 13],
                            scale=1.0,
                        )
                        nc.tensor.matmul(
                            rs_ps,
                            lhsT=ones_bf,
                            rhs=probsT[:, c, :],
                            start=(c == 0),
                            stop=(c == last_c),
                        )
                    # PV: outT[hd, sq_group] accumulated over t-chunks
                    po = psO.tile([P, 512], F32, tag="po")
                    for c in range(last_c + 1):
                        nc.tensor.matmul(
                            po,
                            lhsT=v_all[:, c, j * HD : (j + 1) * HD],
                            rhs=probsT[:, c, :],
                            start=(c == 0),
                            stop=(c == last_c),
                        )
                    # normalize: reciprocal of rowsum, broadcast across
                    # partitions on GpSimd, then one DVE multiply
                    rs_sb = stats.tile([1, 512], F32R, tag="rs_sb")
                    with nc.allow_low_precision(reason="f32r rounding only"):
                        nc.vector.reciprocal(rs_sb, rs_ps)
                    rb = psS.tile([P, 512], F32, tag="rb", bufs=1)
                    nc.tensor.matmul(
                        rb, lhsT=ones1_f, rhs=rs_sb, start=True, stop=True
                    )
                    rb_sb = attn_pool.tile([P, 512], F32, tag="rbsb")
                    nc.vector.tensor_copy(rb_sb, rb)
                    out_sb = attn_pool.tile([P, 512], F32, tag="osb")
                    nc.vector.tensor_tensor(out_sb, po, rb_sb, OP.mult)
                    # + V-projection bias (sum of normalized probs == 1)
                    nc.scalar.activation(
                        out_sb,
                        out_sb,
                        AF.Identity,
                        bias=bvt_sb[:, j : j + 1],
                        scale=1.0,
                    )
                    nc.sync.dma_start(
                        out_d[j][:, G * 512 : (G + 1) * 512], out_sb
                    )

    nc.finalize()
    return nc


_NC_CACHE = None


def _get_nc():
    global _NC_CACHE
    if _NC_CACHE is None:
        _NC_CACHE = build()
    return _NC_CACHE


def _make_in_maps(x, W_kqv, b_kqv):
    x = np.asarray(x, np.float32)
    W = np.asarray(W_kqv, np.float32)
    b = np.asarray(b_kqv, np.float32)
    slopes = _alibi_slopes()
    in_maps = []
    for core in range(8):
        bi, hg = divmod(core, 4)
        heads = [4 * hg + j for j in range(H_LOC)]
        xT = np.ascontiguousarray(x[bi].T).astype(ml_dtypes.bfloat16)
        wkq = np.concatenate(
            [W[:, h * HD : (h + 1) * HD] for h in heads]
            + [W[:, D + h * HD : D + (h + 1) * HD] for h in heads],
            axis=1,
        ).astype(ml_dtypes.bfloat16)
        wv = np.concatenate(
            [W[:, 2 * D + h * HD : 2 * D + (h + 1) * HD] for h in heads], axis=1
        ).astype(ml_dtypes.bfloat16)
        # bias columns: K h0..h3 then Q h0..h3; q-side prescaled by 1/sqrt(hd)
        bkq = np.stack(
            [b[h * HD : (h + 1) * HD] for h in heads]
            + [b[D + h * HD : D + (h + 1) * HD] * SCALE for h in heads],
            axis=1,
        ).astype(np.float32)
        bvt = np.stack(
            [b[2 * D + h * HD : 2 * D + (h + 1) * HD] for h in heads], axis=1
        ).astype(np.float32)  # [hd, H_LOC]
        # biasT[j, tl, sqg] = m_j * (tl - sqg)
        relT = (np.arange(P)[:, None] - np.arange(512)[None, :]).astype(np.float32)
        bias_t = (slopes[heads][:, None, None] * relT[None]).astype(np.float32)
        # negshT[p, j, d+12] = m_j * 128 * d, d in [-12, 3]
        dvals = (np.arange(16) - 12).astype(np.float32) * P
        negsht = np.tile(
            (slopes[heads][:, None] * dvals[None, :])[None], (P, 1, 1)
        ).astype(np.float32)
        # transposed causal: keep tl <= sql
        causalt = np.where(
            np.arange(P)[:, None] <= np.arange(P)[None, :], 0.0, -1e30
        ).astype(np.float32)
        in_maps.append(
            dict(
                xT=xT, wKQ=wkq, wV=wv, bKQ=bkq, bVT=bvt,
                biasT=bias_t, negshT=negsht, causalT=causalt,
            )
        )
    return in_maps


def run(inputs, trace=False, **kw):
    nc = _get_nc()
    in_maps = _make_in_maps(inputs["x"], inputs["W_kqv"], inputs["b_kqv"])
    bkr = run_bass_kernel_spmd(nc, in_maps, core_ids=list(range(8)), trace=trace, **kw)
    B = 2
    out = np.empty((B, NUM_HEADS, S, HD), np.float32)
    for core in range(8):
        bi, hg = divmod(core, 4)
        o = np.asarray(bkr.results[core]["out"])  # [4, 128(hd), 2048(s)]
        for j in range(H_LOC):
            out[bi, 4 * hg + j] = o[j].T
    return out, bkr


def kernel(x, W_kqv, b_kqv):
    out, _ = run({"x": x, "W_kqv": W_kqv, "b_kqv": b_kqv})
    return out
